# revision 88
# baseline (speedup 1.0000x reference)
"""Trainium2 Bass kernel for nn_BasicBlock_81166291960009.

Spatially-gated residual BasicBlock (topk_masking):
  logit_i = conv(x, mask_i_w) + mask_i_b        (64->1ch, 3x3)
  m_i = sigmoid(logit_i) * (logit_i >= 0)
  nm_i = gauss3x3(m_i, sigma_i)
  out1 = relu(bn1(conv1(x))) * nm1
  out  = relu(bn2(conv2(out1)) * nm2 + x)

Sharding: data-parallel, one sample per NeuronCore (N=8 over 8 cores),
weights replicated. No cross-core communication.

V3 layout (per core, C=64, H=W=128):
 - x streamed once: ring batches feed (a) x2 padded conv operand
   ([xhi row-shifted ; xhi], 130x130 zero border) and (b) the exact
   mask tap matmuls: ONE M=36 matmul per 512-pos chunk (lhsT packs
   [mwhi;mwlo] and [mwlo;mwhi] tap columns; a DVE fold adds the two
   halves) -> fp32 tap strip.  conv1 (6 bf16 matmuls/chunk, fp32 PSUM)
   runs in the same pass, evicted UNGATED (relu+bn-bias) to o2 bf16.
 - mask assembly: 9 paired shift-fold DMAs (mask1+mask2 taps share
   shifts), fp32 DVE tree -> logits; thresholding uses the exact fp32
   logit sign.  Gaussian: single-bf16 vertical matmul + fp32 DVE
   horizontal; nm rows stored bf16.
 - conv2 phase: o2 gated in place quarter-ahead (Pool partition
   broadcast of nm1 + Pool multiply; row-shifted dup refreshed by DMA),
   conv2 6 matmuls/chunk + K=1 selector broadcast of nm2, eviction
   bias -> *nm2 -> +residual(fp16) -> relu -> batched output DMAs.
 - SBUF arena: one fp32 [128, NPAD] tile hosts the tap strip
   (partitions 0-17), the bf16 nm rows (bitcast, partitions 0-1 after
   the strip dies), and the fp16 residual (bitcast, partitions 64-127).
 - BN folded into conv weights/bias host-side; all weights arrive in
   2 packed DMAs.
"""

import os
import sys

for _p in ("/opt/trn_rl_repo", "/root/.axon_site/_ro/trn_rl_repo"):
    if os.path.isdir(_p) and _p not in sys.path:
        sys.path.append(_p)

import numpy as np
import ml_dtypes

import concourse.bass as bass
import concourse.bacc as bacc
import concourse.tile as tile
import concourse.mybir as mybir
from concourse import bass_utils

dt = mybir.dt
AF = mybir.ActivationFunctionType
ALU = mybir.AluOpType
BF16 = ml_dtypes.bfloat16
F16 = np.float16

C = 64            # channels
H = W = 128       # spatial
P = 130           # padded pitch
NPOS = H * W      # 16384
NPAD = P * P      # 16900
CHUNK = 512       # positions per PSUM bank (4 image rows)
NCHUNK = NPOS // CHUNK   # 32
EPS_BN = 1e-5
N_CORES = 8

# wb blob column offsets (bf16)
WB_W1P, WB_W1S, WB_W2P, WB_W2S = 0, 192, 384, 576
WB_MW, WB_SEL, WB_SY = 768, 832, 896
WB_COLS = 1152

_COMPILED = None


def _bf16_split(a):
    hi = a.astype(BF16)
    lo = (a.astype(np.float32) - hi.astype(np.float32)).astype(BF16)
    return hi, lo


def _build():
    """Build + compile the single-core Bass module (shapes fixed)."""
    nc = bacc.Bacc("TRN2", target_bir_lowering=False, debug=False,
                   num_devices=N_CORES)

    f32, bf = dt.float32, dt.bfloat16

    xstack_d = nc.dram_tensor("xstack", [2 * C, H, W], dt.float16,
                              kind="ExternalInput")
    xbf_d = nc.dram_tensor("xbf", [C, H, W], bf, kind="ExternalInput")
    mw_d = nc.dram_tensor("mw", [2 * C, 64], dt.float16,
                          kind="ExternalInput")
    wb_d = nc.dram_tensor("wb", [2 * C, WB_COLS], bf, kind="ExternalInput")
    fb_d = nc.dram_tensor("fb", [2 * C, 8], f32, kind="ExternalInput")
    out_d = nc.dram_tensor("out", [C, H, W], f32, kind="ExternalOutput")
    # nm rows staged in DRAM: broadcasts are then plain DMAs with
    # stride-0 DRAM sources (partition_broadcast races in-flight writes)
    nmd_d = nc.dram_tensor("nmd", [2, NPOS], bf, kind="Internal")
    d = dict(xstack_d=xstack_d, xbf_d=xbf_d, mw_d=mw_d, wb_d=wb_d,
             fb_d=fb_d, out_d=out_d, nmd_d=nmd_d)
    if os.environ.get("K_DEBUG") == "1":
        d["dbg_o2"] = nc.dram_tensor("dbg_o2", [2 * C, NPAD], bf,
                                     kind="ExternalOutput")
        d["dbg_nm"] = nc.dram_tensor("dbg_nm", [2, NPOS], bf,
                                     kind="ExternalOutput")
        d["dbg_logit"] = nc.dram_tensor("dbg_logit", [H, 2 * H], f32,
                                        kind="ExternalOutput")

    with tile.TileContext(nc) as tc:
        _emit(nc, tc, d)
    nc.compile()
    return nc


def _emit(nc, tc, d):
    f32, bf, f16 = dt.float32, dt.bfloat16, dt.float16
    from contextlib import ExitStack
    ctx = ExitStack()

    big = ctx.enter_context(tc.tile_pool(name="big", bufs=1))
    wts = ctx.enter_context(tc.tile_pool(name="wts", bufs=1))
    ring = ctx.enter_context(tc.tile_pool(name="ring", bufs=2))
    stg = ctx.enter_context(tc.tile_pool(name="stg", bufs=2))
    ops = ctx.enter_context(tc.tile_pool(name="ops", bufs=1))
    psA = ctx.enter_context(tc.tile_pool(name="psA", bufs=4, space="PSUM"))
    psM = ctx.enter_context(tc.tile_pool(name="psM", bufs=2, space="PSUM"))

    # ---- persistent tiles ----
    x2 = big.tile([2 * C, NPAD], bf, tag="x2")        # padded x + shifted dup
    o2 = big.tile([2 * C, NPAD], bf, tag="o2")        # padded out1 + dup
    arena = big.tile([2 * C, NPAD], f32, tag="arena")
    maps = big.tile([H, 9 * 2 * H], f32, tag="maps")  # [128h, 9tap, 2mask, 128w]

    strip = arena[0:18, :]
    nmd = d["nmd_d"].ap()   # row 0 = nm2, row 1 = nm1 (DRAM)

    wb = wts.tile([2 * C, WB_COLS], bf, tag="wb")
    mw64t = wts.tile([2 * C, 64], f16, tag="mw64t")
    nc.sync.dma_start(mw64t[:], d["mw_d"].ap())
    fb = wts.tile([2 * C, 8], f32, tag="fb")

    w1p = wb[:, WB_W1P:WB_W1P + 3 * C]
    w1s = wb[:, WB_W1S:WB_W1S + 3 * C]
    w2p = wb[:, WB_W2P:WB_W2P + 3 * C]
    w2s = wb[:, WB_W2S:WB_W2S + 3 * C]
    mw64 = mw64t[:, :]
    sel1 = wb[0:1, WB_SEL:WB_SEL + C]
    syh = wb[:, WB_SY:WB_SY + 2 * H]
    b1dup = fb[:, 0:1]
    b2dup = fb[:, 1:2]
    mb = fb[:, 2:4]
    grat = fb[:, 4:6]

    nc.scalar.dma_start(wb[:], d["wb_d"].ap())
    nc.scalar.dma_start(fb[:], d["fb_d"].ap())

    x2v = x2.rearrange("p (r c) -> p r c", c=P)
    o2v = o2.rearrange("p (r c) -> p r c", c=P)
    sv = strip.rearrange("p (r c) -> p r c", c=P)
    mapsv = maps.rearrange("p (t m c) -> p t m c", t=9, m=2)

    # ---- pad memsets (zero borders) ----
    for tv in (x2v, o2v):
        nc.vector.memset(tv[:, 0, :], 0)
        nc.vector.memset(tv[:, P - 1, :], 0)
        nc.vector.memset(tv[:, 1:P - 1, 0:1], 0)
        nc.vector.memset(tv[:, 1:P - 1, P - 1:P], 0)
        nc.vector.memset(tv[C:2 * C, P - 2, :], 0)
    nc.vector.memset(sv[:, 0, :], 0)
    nc.vector.memset(sv[:, P - 1, :], 0)
    nc.vector.memset(sv[:, 1:P - 1, 0:1], 0)
    nc.vector.memset(sv[:, 1:P - 1, P - 1:P], 0)

    # residual: loaded in 4 pieces mid-conv1 (keeps DMA engines free early)

    # ---- mask-assembly tiles; each is written in two row-halves ----
    # (the tap tree accumulates in place inside `maps`)
    logit = ops.tile([H, 2 * H], f32, tag="logit")
    logitv = logit.rearrange("p (m c) -> p m c", m=2)
    pp = ops.tile([H, 2 * H], f32, tag="pp")
    mbf = ops.tile([H, 2 * H], bf, tag="mbf")
    mbfv = mbf.rearrange("p (m c) -> p m c", m=2)
    nmv = ops.tile([H, 2 * P], f32, tag="nmv")
    nmvv = nmv.rearrange("p (m c) -> p m c", m=2)
    t2 = ops.tile([H, 2 * H], f32, tag="t2")
    t2v = t2.rearrange("p (m c) -> p m c", m=2)
    nmf = ops.tile([H, 2 * H], bf, tag="nmf")
    nmfv = nmf.rearrange("p (m c) -> p m c", m=2)
    # the K=128 vertical-gauss of each half multiplies the other half's
    # (possibly unwritten) mbf rows by zero sy-weights; keep them finite
    nc.vector.memset(mbf[:], 0)
    nc.vector.memset(nmvv[:, :, 0:1], 0)
    nc.vector.memset(nmvv[:, :, P - 1:P], 0)

    engs = (nc.sync, nc.scalar, nc.gpsimd)

    def maps_half(top):
        # sync/scalar only: a gpsimd DMA here would convoy ahead of the
        # Pool-engine broadcasts that conv2 is waiting on
        r0, r1 = (0, 96) if top else (96, 128)
        for t in range(18):
            t9, mi = t % 9, t // 9
            dy, dx = t9 // 3 - 1, t9 % 3 - 1
            src = sv[9 * mi + t9:9 * mi + t9 + 1,
                     1 + dy + r0:1 + dy + r1, 1 + dx:129 + dx]
            engs[t % 2].dma_start(mapsv[r0:r1, t9, mi, :], src)

    def half_closures(top):
        # logit/m rows split at 96 (SBUF partition bases must be 32-aligned);
        # gauss/nm output rows split at 64 (each K=128 gauss reads the m rows
        # it needs from either piece)
        r0, r1 = (0, 96) if top else (96, 128)
        g0, g1 = (0, C) if top else (C, 2 * C)
        box = {}

        def op_u1():
            nc.vector.tensor_add(maps[r0:r1, 0:4 * 2 * H],
                                 maps[r0:r1, 0:4 * 2 * H],
                                 maps[r0:r1, 4 * 2 * H:8 * 2 * H])

        def op_u2():
            nc.vector.tensor_add(maps[r0:r1, 0:2 * 2 * H],
                                 maps[r0:r1, 0:2 * 2 * H],
                                 maps[r0:r1, 2 * 2 * H:4 * 2 * H])

        def op_u3():
            nc.vector.tensor_add(maps[r0:r1, 0:2 * H],
                                 maps[r0:r1, 0:2 * H],
                                 maps[r0:r1, 2 * H:4 * H])

        def op_logit():
            for mi in range(2):
                nc.vector.scalar_tensor_tensor(
                    logitv[r0:r1, mi, :], mapsv[r0:r1, 0, mi, :],
                    mb[r0:r1, mi:mi + 1], mapsv[r0:r1, 8, mi, :],
                    op0=ALU.add, op1=ALU.add)

        def op_sig():
            nc.scalar.activation(pp[r0:r1, :], logit[r0:r1, :], AF.Sigmoid)

        def op_m():
            # hard mask in place over the sigmoid values
            nc.vector.scalar_tensor_tensor(
                pp[r0:r1, :], logit[r0:r1, :], 0.0, pp[r0:r1, :],
                op0=ALU.is_ge, op1=ALU.mult)

        def op_mbf():
            nc.vector.tensor_copy(mbf[r0:r1, :], pp[r0:r1, :])

        def op_gauss():
            png = psM.tile([128, CHUNK], f32, tag="b")
            pngv = png[:, 0:2 * H].rearrange("p (m c) -> p m c", m=2)
            box["pngv"] = pngv
            for mi in range(2):
                nc.tensor.matmul(pngv[g0:g1, mi, :],
                                 syh[:, mi * H + g0:mi * H + g1],
                                 mbfv[:, mi, :], start=True, stop=True)

        def op_nmvc():
            nc.scalar.copy(nmvv[g0:g1, :, 1:129], box["pngv"][g0:g1, :, :])

        def op_t2():
            nc.vector.tensor_add(t2v[g0:g1, :, :], nmvv[g0:g1, :, 0:128],
                                 nmvv[g0:g1, :, 2:130])

        def op_nm():
            for mi in range(2):
                nc.vector.scalar_tensor_tensor(
                    nmfv[g0:g1, mi, :], t2v[g0:g1, mi, :],
                    grat[g0:g1, mi:mi + 1], nmvv[g0:g1, mi, 1:129],
                    op0=ALU.mult, op1=ALU.add)

        def op_rows():
            nv = nmd.rearrange("p (h w) -> p h w", w=W)
            nc.sync.dma_start(nv[0:1, g0:g1, :], nmfv[g0:g1, 1, :])
            nc.scalar.dma_start(nv[1:2, g0:g1, :], nmfv[g0:g1, 0, :])

        return [op_u1, op_u2, op_u3, op_logit, op_sig, op_m, op_mbf,
                op_gauss, op_nmvc, op_t2, op_nm, op_rows]

    def gate_slab(s):
        # gates chunks 2s, 2s+1 (rows 8s..8s+7); nm1 broadcast to all 128
        # partitions by a stride-0-source DMA from DRAM
        nm1b = stg.tile([2 * C, 2 * CHUNK], bf, tag="nm1b")
        sl = nmd[1:2, s * 2 * CHUNK:(s + 1) * 2 * CHUNK]
        bsrc = bass.AP(sl.tensor, sl.offset, [[0, 2 * C]] + list(sl.ap[1:]))
        (nc.sync, nc.scalar)[s % 2].dma_start(nm1b[:], bsrc)
        nmv8lo = nm1b[0:C, :].rearrange("p (r c) -> p r c", c=W)
        nmv8up = nm1b[C:2 * C, :].rearrange("p (r c) -> p r c", c=W)
        r0 = 8 * s
        lo = o2v[0:C, r0 + 1:r0 + 9, 1:129]
        nc.vector.tensor_mul(lo, lo, nmv8lo)
        up = o2v[C:2 * C, r0:r0 + 8, 1:129]
        nc.vector.tensor_mul(up, up, nmv8up)

    # =====================================================================
    # conv helper (baseline-proven indexing)
    # =====================================================================
    def conv_chunk(src2v, wp, ws, ps, par, k):
        tp = (0, 0) if par == 0 else (0, 64)
        po = ps[0:C, :] if par == 0 else ps[C:2 * C, :]
        r0 = 4 * k + 1
        for kx in range(3):
            dx = kx - 1
            rhs = src2v[:, r0 - 1:r0 + 3, 1 + dx:129 + dx]
            nc.tensor.matmul(po, wp[:, kx * C:(kx + 1) * C], rhs,
                             start=(kx == 0), stop=False, tile_position=tp)
            rhs1 = src2v[:, r0 + 1:r0 + 5, 1 + dx:129 + dx]
            nc.tensor.matmul(po, ws[:, kx * C:(kx + 1) * C], rhs1,
                             start=False, stop=(kx == 2), tile_position=tp)

    # =====================================================================
    # Phase 1: stream x; mask taps (1 matmul/chunk) + conv1 (ungated)
    # =====================================================================
    pA = None

    def conv1_chunk(k):
        nonlocal pA
        par = k % 2
        if par == 0:
            pA = psA.tile([2 * C, CHUNK], f32, tag="cv")
        conv_chunk(x2v, w1p, w1s, pA, par, k)
        if par == 1:
            for pr in range(2):
                kk = k - 1 + pr
                rr = 4 * kk + 1
                h0, h1 = (0, C) if pr == 0 else (C, 2 * C)
                dst = o2v[0:C, rr:rr + 4, 1:129]
                nc.scalar.activation(dst, pA[h0:h1, :], AF.Relu,
                                     bias=b1dup[h0:h1, :])
        # refresh the row-shifted dup (UNGATED; gating hits both halves
        # later) as soon as a 32-row quarter is fully evicted
        if k >= 10 and (k - 10) % 8 == 0:
            q = (k - 10) // 8
            nc.gpsimd.dma_start(o2v[C:2 * C, 32 * q:32 * q + 32, 1:129],
                                o2v[0:C, 32 * q + 1:32 * q + 33, 1:129])

    top_list = half_closures(True)
    xsf = d["xstack_d"].ap().rearrange("c h w -> c (h w)")
    scr0 = ops.tile([1, 1], f32, tag="scr0")
    scr1 = ops.tile([1, 1], f32, tag="scr1")
    for kb in range(NCHUNK // 4):
        if kb == 1:
            # preload the sigmoid table once the first fills are queued
            nc.vector.memset(scr0[:], 0)
            nc.scalar.activation(scr1[:], scr0[:], AF.Sigmoid)
        xst = ring.tile([2 * C, 4 * CHUNK], f16, tag="xst")
        nc.sync.dma_start(xst[:], xsf[:, kb * 4 * CHUNK:(kb + 1) * 4 * CHUNK])
        rb = 16 * kb
        # conv operand filled from the bf16 copy of x straight from DRAM
        xbsrc = d["xbf_d"].ap().rearrange("c h w -> c h w")[:, rb:rb + 16, :]
        nc.scalar.dma_start(x2v[0:C, rb + 1:rb + 17, 1:129], xbsrc)
        nc.gpsimd.dma_start(x2v[C:2 * C, rb:rb + 16, 1:129], xbsrc)
        for j in range(4):
            k = 4 * kb + j
            r0 = 4 * k + 1
            psm = psM.tile([128, CHUNK], f32, tag="m")
            po = psm[0:64, :]
            nc.tensor.matmul(po, mw64[:], xst[:, j * CHUNK:(j + 1) * CHUNK],
                             start=True, stop=True, tile_position=(0, 0))
            # B-terms sit at partitions 32..49 (PSUM reads must start
            # 32-aligned); only one PSUM operand allowed per DVE op, so
            # stage them through SBUF on the Act engine.
            sb18 = stg.tile([18, CHUNK], f32, tag="sb18")
            nc.scalar.copy(sb18[:], psm[32:50, :])
            nc.vector.tensor_add(sv[:, r0:r0 + 4, 1:129],
                                 psm[0:18, :].rearrange("p (r c) -> p r c", c=W),
                                 sb18[:].rearrange("p (r c) -> p r c", c=W))
            if k == 24:
                # strip rows 0..97 final -> top-piece mask assembly can start
                maps_half(True)
            if k >= 25 and top_list:
                top_list.pop(0)()
            if k >= 2:
                conv1_chunk(k - 2)
    conv1_chunk(NCHUNK - 2)
    conv1_chunk(NCHUNK - 1)
    nc.gpsimd.dma_start(o2v[C:2 * C, 96:128, 1:129],
                        o2v[0:C, 97:129, 1:129])
    while top_list:
        top_list.pop(0)()

    # top-half gates can run during the conv1 tail / before conv2
    for s in range(3):
        gate_slab(s)

    # =====================================================================
    # Phase 2/3: bottom-half assembly overlapped with conv2 + evict
    # =====================================================================
    maps_half(False)
    bot_list = half_closures(False)

    # out pair view: partition u*64+c, free (j, w) -> c*NPOS + (2j+u)*512 + w
    outf2 = d["out_d"].ap().rearrange("c h w -> c (h w)").rearrange(
        "c (j u w) -> u c j w", u=2, w=CHUNK)

    # slab s gated before conv2 chunk (8s-5)/4 needs it; slabs >= 8 need
    # the bottom-half nm rows (ready ~chunk 8); spread 1-per-2-chunks to
    # keep the DVE gate muls from spiking
    SLAB_AT = {0: 3, 2: 4, 4: 5, 6: 6, 8: 7, 10: 8, 12: 9, 14: 10, 16: 11,
               18: 12, 20: 13, 22: 14, 24: 15}

    pB = None
    nm2b = None
    ost = None
    for k in range(NCHUNK):
        if bot_list:
            bot_list.pop(0)()
            if k < 4 and bot_list:
                bot_list.pop(0)()
        if k in SLAB_AT:
            gate_slab(SLAB_AT[k])
        par = k % 2
        if par == 0:
            ost = stg.tile([2 * C, CHUNK], f32, tag="ost")
            pB = psA.tile([2 * C, CHUNK], f32, tag="cv")
            # nm2 pair broadcast: partitions 0-63 = chunk k, 64-127 = k+1
            nm2b = stg.tile([2 * C, CHUNK], bf, tag="nm2b")
            sl = nmd[0:1, k * CHUNK:(k + 1) * CHUNK]
            bsrc = bass.AP(sl.tensor, sl.offset,
                           [[CHUNK, 2], [0, C]] + list(sl.ap[1:]))
            (nc.sync, nc.scalar)[(k // 2) % 2].dma_start(nm2b[:], bsrc)
        conv_chunk(o2v, w2p, w2s, pB, par, k)
        if par == 1:
            j = k // 2
            # (conv2 + bn-bias) * nm2 in one DVE op straight from PSUM
            nc.vector.scalar_tensor_tensor(ost[:], pB[:], b2dup[:], nm2b[:],
                                           op0=ALU.add, op1=ALU.mult)
            # residual (bf16 xhi) read straight from x2's two halves:
            # lower half slot r+1 = row r (even chunk), upper slot r = row r
            # (both on DVE: gpsimd elementwise mis-addresses strided APs)
            ke = k - 1
            nc.vector.tensor_add(ost[0:C, :], ost[0:C, :],
                                 x2v[0:C, 4 * ke + 1:4 * ke + 5, 1:129])
            nc.vector.tensor_add(ost[C:2 * C, :], ost[C:2 * C, :],
                                 x2v[C:2 * C, 4 * k:4 * k + 4, 1:129])
            nc.scalar.activation(ost[:], ost[:], AF.Relu)
            eng = (nc.sync, nc.scalar)[j % 2]
            eng.dma_start(outf2[:, :, j, :], ost[:])

    if "dbg_o2" in d:
        nc.sync.dma_start(d["dbg_o2"].ap(), o2[:])
        nc.sync.dma_start(d["dbg_nm"].ap()[0:1, :], nmd[1:2, :])
        nc.sync.dma_start(d["dbg_nm"].ap()[1:2, :], nmd[0:1, :])
        nc.sync.dma_start(d["dbg_logit"].ap(), logit[:])

    ctx.close()


def _host_prep(inputs):
    """Fold BN, split dtypes, build packed weight blobs."""
    f32 = np.float32
    x = np.asarray(inputs["x"], f32)
    N = x.shape[0]

    def fold(w, gamma, beta, mean, var):
        scale = (np.asarray(gamma, f32)
                 / np.sqrt(np.asarray(var, f32) + f32(EPS_BN)))
        wf = np.asarray(w, f32) * scale[:, None, None, None]
        b = np.asarray(beta, f32) - np.asarray(mean, f32) * scale
        return wf, b

    w1f, b1 = fold(inputs["conv1_w"], inputs["bn1_gamma"], inputs["bn1_beta"],
                   inputs["bn1_mean"], inputs["bn1_var"])
    w2f, b2 = fold(inputs["conv2_w"], inputs["bn2_gamma"], inputs["bn2_beta"],
                   inputs["bn2_mean"], inputs["bn2_var"])

    def pack_conv(wf):
        wp = np.zeros((3, 2 * C, C), f32)
        wsg = np.zeros((3, 2 * C, C), f32)
        for kx in range(3):
            wp[kx, 0:C] = wf[:, :, 0, kx].T      # ky=0 (dy=-1), lower half
            wp[kx, C:2 * C] = wf[:, :, 1, kx].T  # ky=1 (dy=0) via shifted dup
            wsg[kx, 0:C] = wf[:, :, 2, kx].T     # ky=2 (dy=+1); upper half 0
        # [3, 2C, C] -> [2C, 3C]
        return (wp.transpose(1, 0, 2).reshape(2 * C, 3 * C),
                wsg.transpose(1, 0, 2).reshape(2 * C, 3 * C))

    w1p, w1s = pack_conv(w1f)
    w2p, w2s = pack_conv(w2f)

    mw = np.zeros((C, 18), f32)
    for t in range(9):
        ky, kx = t // 3, t % 3
        mw[:, t] = np.asarray(inputs["mask1_w"], f32)[0, :, ky, kx]
        mw[:, 9 + t] = np.asarray(inputs["mask2_w"], f32)[0, :, ky, kx]
    mwhi = mw.astype(F16).astype(f32)
    mwlo = (mw - mwhi).astype(F16).astype(f32)
    mw64 = np.zeros((2 * C, 64), f32)
    mw64[0:C, 0:18] = mwhi
    mw64[C:2 * C, 0:18] = mwlo
    mw64[0:C, 32:50] = mwlo
    mw64[C:2 * C, 32:50] = mwhi
    mw64 = mw64.astype(F16)

    syh = np.zeros((2 * C, 2 * H), f32)
    grat = np.zeros((2, H), f32)
    for mi, sig in enumerate((inputs["sigma1"], inputs["sigma2"])):
        s = f32(np.asarray(sig, f32).reshape(-1)[0])
        dd = np.arange(3, dtype=f32) - f32(1.0)
        e = np.exp(-(dd * dd) / (2 * s * s)).astype(f32)
        g1d = (e / e.sum()).astype(f32)
        g0, g1 = g1d[1], g1d[0]
        sy = np.zeros((H, H), f32)
        for h in range(H):
            for dyy in (-1, 0, 1):
                hh = h + dyy
                if 0 <= hh < H:
                    sy[hh, h] = g0 * g1d[dyy + 1]   # lhsT[h_in, h_out]
        syh[:, mi * H:(mi + 1) * H] = sy
        grat[mi, :] = g1 / g0

    wb = np.zeros((2 * C, WB_COLS), f32)
    wb[:, WB_W1P:WB_W1P + 3 * C] = w1p
    wb[:, WB_W1S:WB_W1S + 3 * C] = w1s
    wb[:, WB_W2P:WB_W2P + 3 * C] = w2p
    wb[:, WB_W2S:WB_W2S + 3 * C] = w2s

    wb[0, WB_SEL:WB_SEL + C] = 1.0
    wb[:, WB_SY:WB_SY + 2 * H] = syh
    wb = wb.astype(BF16)

    fb = np.zeros((2 * C, 8), f32)
    fb[:, 0] = np.concatenate([b1, b1])
    fb[:, 1] = np.concatenate([b2, b2])
    fb[:, 2] = np.asarray(inputs["mask1_b"], f32).reshape(-1)[0]
    fb[:, 3] = np.asarray(inputs["mask2_b"], f32).reshape(-1)[0]
    fb[0:H, 4] = grat[0]
    fb[0:H, 5] = grat[1]

    per_core = []
    for i in range(N):
        xi = x[i]
        xhi = xi.astype(F16)
        xlo = (xi - xhi.astype(f32)).astype(F16)
        xstack = np.concatenate([xhi, xlo], axis=0)
        per_core.append(dict(
            xstack=xstack,
            xbf=xi.astype(BF16),
            mw=mw64,
            wb=wb, fb=fb,
        ))
    return per_core


def kernel(**inputs):
    global _COMPILED
    if _COMPILED is None:
        _COMPILED = _build()
    nc = _COMPILED
    per_core = _host_prep(inputs)
    res = bass_utils.run_bass_kernel_spmd(
        nc, per_core, core_ids=list(range(len(per_core))))
    out = np.stack([res.results[i]["out"] for i in range(len(per_core))])
    return out.astype(np.float32)


if __name__ == "__main__":
    sys.path.insert(0, os.path.dirname(os.path.abspath(__file__)))
    import jax
    import reference
    with jax.default_device(jax.devices("cpu")[0]):
        ins = {k: np.asarray(v) for k, v in reference.setup_inputs().items()}
        exp = np.asarray(reference.reference(**ins))
    act = kernel(**ins)
    err = np.abs(act - exp)
    denom = np.abs(exp).max()
    print(f"max abs err: {err.max():.3e}  (ref scale {denom:.3f})")
    print(f"Relative error: {err.max() / denom:.3e}")


# revision 96
# speedup vs baseline: 1.0551x; 1.0551x over previous
"""Trainium2 Bass kernel for nn_BasicBlock_81166291960009.

Spatially-gated residual BasicBlock (topk_masking):
  logit_i = conv(x, mask_i_w) + mask_i_b        (64->1ch, 3x3)
  m_i = sigmoid(logit_i) * (logit_i >= 0)
  nm_i = gauss3x3(m_i, sigma_i)
  out1 = relu(bn1(conv1(x))) * nm1
  out  = relu(bn2(conv2(out1)) * nm2 + x)

Sharding: data-parallel, one sample per NeuronCore (N=8 over 8 cores),
weights replicated. No cross-core communication.

V3 layout (per core, C=64, H=W=128):
 - x streamed once: ring batches feed (a) x2 padded conv operand
   ([xhi row-shifted ; xhi], 130x130 zero border) and (b) the exact
   mask tap matmuls: ONE M=36 matmul per 512-pos chunk (lhsT packs
   [mwhi;mwlo] and [mwlo;mwhi] tap columns; a DVE fold adds the two
   halves) -> fp32 tap strip.  conv1 (6 bf16 matmuls/chunk, fp32 PSUM)
   runs in the same pass, evicted UNGATED (relu+bn-bias) to o2 bf16.
 - mask assembly: 9 paired shift-fold DMAs (mask1+mask2 taps share
   shifts), fp32 DVE tree -> logits; thresholding uses the exact fp32
   logit sign.  Gaussian: single-bf16 vertical matmul + fp32 DVE
   horizontal; nm rows stored bf16.
 - conv2 phase: o2 gated in place quarter-ahead (Pool partition
   broadcast of nm1 + Pool multiply; row-shifted dup refreshed by DMA),
   conv2 6 matmuls/chunk + K=1 selector broadcast of nm2, eviction
   bias -> *nm2 -> +residual(fp16) -> relu -> batched output DMAs.
 - SBUF arena: one fp32 [128, NPAD] tile hosts the tap strip
   (partitions 0-17), the bf16 nm rows (bitcast, partitions 0-1 after
   the strip dies), and the fp16 residual (bitcast, partitions 64-127).
 - BN folded into conv weights/bias host-side; all weights arrive in
   2 packed DMAs.
"""

import os
import sys

for _p in ("/opt/trn_rl_repo", "/root/.axon_site/_ro/trn_rl_repo"):
    if os.path.isdir(_p) and _p not in sys.path:
        sys.path.append(_p)

import numpy as np
import ml_dtypes

import concourse.bass as bass
import concourse.bacc as bacc
import concourse.tile as tile
import concourse.mybir as mybir
from concourse import bass_utils

dt = mybir.dt
AF = mybir.ActivationFunctionType
ALU = mybir.AluOpType
BF16 = ml_dtypes.bfloat16
F16 = np.float16

C = 64            # channels
H = W = 128       # spatial
P = 130           # padded pitch
NPOS = H * W      # 16384
NPAD = P * P      # 16900
CHUNK = 512       # positions per PSUM bank (4 image rows)
NCHUNK = NPOS // CHUNK   # 32
EPS_BN = 1e-5
N_CORES = 8

# wb blob column offsets (bf16)
WB_W1P, WB_W1S, WB_W2P, WB_W2S = 0, 192, 384, 576
WB_MW, WB_SEL, WB_SY = 768, 832, 896
WB_COLS = 1152

_COMPILED = None


def _bf16_split(a):
    hi = a.astype(BF16)
    lo = (a.astype(np.float32) - hi.astype(np.float32)).astype(BF16)
    return hi, lo


def _build():
    """Build + compile the single-core Bass module (shapes fixed)."""
    nc = bacc.Bacc("TRN2", target_bir_lowering=False, debug=False,
                   num_devices=N_CORES)

    f32, bf = dt.float32, dt.bfloat16

    xstack_d = nc.dram_tensor("xstack", [2 * C, H, W], dt.float16,
                              kind="ExternalInput")
    xbf_d = nc.dram_tensor("xbf", [C, H, W], bf, kind="ExternalInput")
    mw_d = nc.dram_tensor("mw", [2 * C, 64], dt.float16,
                          kind="ExternalInput")
    wb_d = nc.dram_tensor("wb", [2 * C, WB_COLS], bf, kind="ExternalInput")
    fb_d = nc.dram_tensor("fb", [2 * C, 8], f32, kind="ExternalInput")
    out_d = nc.dram_tensor("out", [C, H, W], f32, kind="ExternalOutput")
    # nm rows staged in DRAM: broadcasts are then plain DMAs with
    # stride-0 DRAM sources (partition_broadcast races in-flight writes)
    nmd_d = nc.dram_tensor("nmd", [2, NPOS], bf, kind="Internal")
    d = dict(xstack_d=xstack_d, xbf_d=xbf_d, mw_d=mw_d, wb_d=wb_d,
             fb_d=fb_d, out_d=out_d, nmd_d=nmd_d)
    if os.environ.get("K_DEBUG") == "1":
        d["dbg_o2"] = nc.dram_tensor("dbg_o2", [2 * C, NPAD], bf,
                                     kind="ExternalOutput")
        d["dbg_nm"] = nc.dram_tensor("dbg_nm", [2, NPOS], bf,
                                     kind="ExternalOutput")
        d["dbg_logit"] = nc.dram_tensor("dbg_logit", [H, 2 * H], f32,
                                        kind="ExternalOutput")

    with tile.TileContext(nc) as tc:
        _emit(nc, tc, d)
    nc.compile()
    return nc


def _emit(nc, tc, d):
    f32, bf, f16 = dt.float32, dt.bfloat16, dt.float16
    from contextlib import ExitStack
    ctx = ExitStack()

    big = ctx.enter_context(tc.tile_pool(name="big", bufs=1))
    wts = ctx.enter_context(tc.tile_pool(name="wts", bufs=1))
    ring = ctx.enter_context(tc.tile_pool(name="ring", bufs=2))
    stg = ctx.enter_context(tc.tile_pool(name="stg", bufs=2))
    ops = ctx.enter_context(tc.tile_pool(name="ops", bufs=1))
    psA = ctx.enter_context(tc.tile_pool(name="psA", bufs=4, space="PSUM"))
    psM = ctx.enter_context(tc.tile_pool(name="psM", bufs=2, space="PSUM"))

    # ---- persistent tiles ----
    x2 = big.tile([2 * C, NPAD], bf, tag="x2")        # padded x + shifted dup
    o2 = big.tile([2 * C, NPAD], bf, tag="o2")        # padded out1 + dup
    arena = big.tile([2 * C, NPAD], f32, tag="arena")
    maps = big.tile([H, 9 * 2 * H], f32, tag="maps")  # [128h, 9tap, 2mask, 128w]

    strip = arena[0:18, :]
    nmd = d["nmd_d"].ap()   # row 0 = nm2, row 1 = nm1 (DRAM)

    wb = wts.tile([2 * C, WB_COLS], bf, tag="wb")
    mw64t = wts.tile([2 * C, 64], f16, tag="mw64t")
    nc.sync.dma_start(mw64t[:], d["mw_d"].ap())
    fb = wts.tile([2 * C, 8], f32, tag="fb")

    w1p = wb[:, WB_W1P:WB_W1P + 3 * C]
    w1s = wb[:, WB_W1S:WB_W1S + 3 * C]
    w2p = wb[:, WB_W2P:WB_W2P + 3 * C]
    w2s = wb[:, WB_W2S:WB_W2S + 3 * C]
    mw64 = mw64t[:, :]
    sel1 = wb[0:1, WB_SEL:WB_SEL + C]
    syh = wb[:, WB_SY:WB_SY + 2 * H]
    b1dup = fb[:, 0:1]
    b2dup = fb[:, 1:2]
    mb = fb[:, 2:4]
    grat = fb[:, 4:6]

    nc.scalar.dma_start(wb[:], d["wb_d"].ap())
    nc.scalar.dma_start(fb[:], d["fb_d"].ap())

    x2v = x2.rearrange("p (r c) -> p r c", c=P)
    o2v = o2.rearrange("p (r c) -> p r c", c=P)
    sv = strip.rearrange("p (r c) -> p r c", c=P)
    mapsv = maps.rearrange("p (t m c) -> p t m c", t=9, m=2)

    # ---- pad memsets (zero borders) ----
    for tv in (x2v, o2v):
        nc.vector.memset(tv[:, 0, :], 0)
        nc.vector.memset(tv[:, P - 1, :], 0)
        nc.vector.memset(tv[:, 1:P - 1, 0:1], 0)
        nc.vector.memset(tv[:, 1:P - 1, P - 1:P], 0)
        nc.vector.memset(tv[C:2 * C, P - 2, :], 0)
    nc.vector.memset(sv[:, 0, :], 0)
    nc.vector.memset(sv[:, P - 1, :], 0)
    nc.vector.memset(sv[:, 1:P - 1, 0:1], 0)
    nc.vector.memset(sv[:, 1:P - 1, P - 1:P], 0)

    # residual: loaded in 4 pieces mid-conv1 (keeps DMA engines free early)

    # ---- mask-assembly tiles; each is written in two row-halves ----
    # (the tap tree accumulates in place inside `maps`)
    logit = ops.tile([H, 2 * H], f32, tag="logit")
    logitv = logit.rearrange("p (m c) -> p m c", m=2)
    pp = ops.tile([H, 2 * H], f32, tag="pp")
    mbf = ops.tile([H, 2 * H], bf, tag="mbf")
    mbfv = mbf.rearrange("p (m c) -> p m c", m=2)
    nmv = ops.tile([H, 2 * P], f32, tag="nmv")
    nmvv = nmv.rearrange("p (m c) -> p m c", m=2)
    t2 = ops.tile([H, 2 * H], f32, tag="t2")
    t2v = t2.rearrange("p (m c) -> p m c", m=2)
    nmf = ops.tile([H, 2 * H], bf, tag="nmf")
    nmfv = nmf.rearrange("p (m c) -> p m c", m=2)
    # the K=128 vertical-gauss of each half multiplies the other half's
    # (possibly unwritten) mbf rows by zero sy-weights; keep them finite
    nc.vector.memset(mbf[:], 0)
    nc.vector.memset(nmvv[:, :, 0:1], 0)
    nc.vector.memset(nmvv[:, :, P - 1:P], 0)

    engs = (nc.sync, nc.scalar, nc.gpsimd)

    def maps_half(top):
        # sync/scalar only: a gpsimd DMA here would convoy ahead of the
        # Pool-engine broadcasts that conv2 is waiting on
        r0, r1 = (0, 96) if top else (96, 128)
        for t in range(18):
            t9, mi = t % 9, t // 9
            dy, dx = t9 // 3 - 1, t9 % 3 - 1
            src = sv[9 * mi + t9:9 * mi + t9 + 1,
                     1 + dy + r0:1 + dy + r1, 1 + dx:129 + dx]
            engs[t % 2].dma_start(mapsv[r0:r1, t9, mi, :], src)

    def half_closures(top):
        # logit/m rows split at 96 (SBUF partition bases must be 32-aligned);
        # gauss/nm output rows split at 64 (each K=128 gauss reads the m rows
        # it needs from either piece)
        r0, r1 = (0, 96) if top else (96, 128)
        g0, g1 = (0, C) if top else (C, 2 * C)
        box = {}

        def op_u1():
            nc.vector.tensor_add(maps[r0:r1, 0:4 * 2 * H],
                                 maps[r0:r1, 0:4 * 2 * H],
                                 maps[r0:r1, 4 * 2 * H:8 * 2 * H])

        def op_u2():
            nc.vector.tensor_add(maps[r0:r1, 0:2 * 2 * H],
                                 maps[r0:r1, 0:2 * 2 * H],
                                 maps[r0:r1, 2 * 2 * H:4 * 2 * H])

        def op_u3():
            nc.vector.tensor_add(maps[r0:r1, 0:2 * H],
                                 maps[r0:r1, 0:2 * H],
                                 maps[r0:r1, 2 * H:4 * H])

        def op_logit():
            for mi in range(2):
                nc.vector.scalar_tensor_tensor(
                    logitv[r0:r1, mi, :], mapsv[r0:r1, 0, mi, :],
                    mb[r0:r1, mi:mi + 1], mapsv[r0:r1, 8, mi, :],
                    op0=ALU.add, op1=ALU.add)

        def op_sig():
            nc.scalar.activation(pp[r0:r1, :], logit[r0:r1, :], AF.Sigmoid)

        def op_m():
            # hard mask in place over the sigmoid values
            nc.vector.scalar_tensor_tensor(
                pp[r0:r1, :], logit[r0:r1, :], 0.0, pp[r0:r1, :],
                op0=ALU.is_ge, op1=ALU.mult)

        def op_mbf():
            nc.vector.tensor_copy(mbf[r0:r1, :], pp[r0:r1, :])

        def op_gauss():
            png = psM.tile([128, CHUNK], f32, tag="b")
            pngv = png[:, 0:2 * H].rearrange("p (m c) -> p m c", m=2)
            box["pngv"] = pngv
            for mi in range(2):
                nc.tensor.matmul(pngv[g0:g1, mi, :],
                                 syh[:, mi * H + g0:mi * H + g1],
                                 mbfv[:, mi, :], start=True, stop=True)

        def op_nmvc():
            nc.scalar.copy(nmvv[g0:g1, :, 1:129], box["pngv"][g0:g1, :, :])

        def op_t2():
            nc.vector.tensor_add(t2v[g0:g1, :, :], nmvv[g0:g1, :, 0:128],
                                 nmvv[g0:g1, :, 2:130])

        def op_nm():
            for mi in range(2):
                nc.vector.scalar_tensor_tensor(
                    nmfv[g0:g1, mi, :], t2v[g0:g1, mi, :],
                    grat[g0:g1, mi:mi + 1], nmvv[g0:g1, mi, 1:129],
                    op0=ALU.mult, op1=ALU.add)

        def op_rows():
            nv = nmd.rearrange("p (h w) -> p h w", w=W)
            nc.sync.dma_start(nv[0:1, g0:g1, :], nmfv[g0:g1, 1, :])
            nc.scalar.dma_start(nv[1:2, g0:g1, :], nmfv[g0:g1, 0, :])

        return [op_u1, op_u2, op_u3, op_logit, op_sig, op_m, op_mbf,
                op_gauss, op_nmvc, op_t2, op_nm, op_rows]

    def gate_slab(s):
        # gates chunks 2s, 2s+1 (rows 8s..8s+7); nm1 broadcast to all 128
        # partitions by a stride-0-source DMA from DRAM
        nm1b = stg.tile([2 * C, 2 * CHUNK], bf, tag="nm1b")
        sl = nmd[1:2, s * 2 * CHUNK:(s + 1) * 2 * CHUNK]
        bsrc = bass.AP(sl.tensor, sl.offset, [[0, 2 * C]] + list(sl.ap[1:]))
        (nc.sync, nc.scalar)[s % 2].dma_start(nm1b[:], bsrc)
        nmv8lo = nm1b[0:C, :].rearrange("p (r c) -> p r c", c=W)
        nmv8up = nm1b[C:2 * C, :].rearrange("p (r c) -> p r c", c=W)
        r0 = 8 * s
        lo = o2v[0:C, r0 + 1:r0 + 9, 1:129]
        nc.vector.tensor_mul(lo, lo, nmv8lo)
        up = o2v[C:2 * C, r0:r0 + 8, 1:129]
        nc.vector.tensor_mul(up, up, nmv8up)

    # =====================================================================
    # conv helper (baseline-proven indexing)
    # =====================================================================
    def conv_chunk(src2v, wp, ws, ps, par, k):
        tp = (0, 0) if par == 0 else (0, 64)
        po = ps[0:C, :] if par == 0 else ps[C:2 * C, :]
        r0 = 4 * k + 1
        for kx in range(3):
            dx = kx - 1
            rhs = src2v[:, r0 - 1:r0 + 3, 1 + dx:129 + dx]
            nc.tensor.matmul(po, wp[:, kx * C:(kx + 1) * C], rhs,
                             start=(kx == 0), stop=False, tile_position=tp)
            rhs1 = src2v[:, r0 + 1:r0 + 5, 1 + dx:129 + dx]
            nc.tensor.matmul(po, ws[:, kx * C:(kx + 1) * C], rhs1,
                             start=False, stop=(kx == 2), tile_position=tp)

    # =====================================================================
    # Phase 1: stream x; mask taps (1 matmul/chunk) + conv1 (ungated)
    # =====================================================================
    pA = None

    def conv1_chunk(k):
        nonlocal pA
        par = k % 2
        if par == 0:
            pA = psA.tile([2 * C, CHUNK], f32, tag="cv")
        conv_chunk(x2v, w1p, w1s, pA, par, k)
        if par == 1:
            for pr in range(2):
                kk = k - 1 + pr
                rr = 4 * kk + 1
                h0, h1 = (0, C) if pr == 0 else (C, 2 * C)
                dst = o2v[0:C, rr:rr + 4, 1:129]
                nc.scalar.activation(dst, pA[h0:h1, :], AF.Relu,
                                     bias=b1dup[h0:h1, :])
        # refresh the row-shifted dup (UNGATED; gating hits both halves
        # later) as soon as a 32-row quarter is fully evicted
        if k >= 10 and (k - 10) % 8 == 0:
            q = (k - 10) // 8
            nc.gpsimd.dma_start(o2v[C:2 * C, 32 * q:32 * q + 32, 1:129],
                                o2v[0:C, 32 * q + 1:32 * q + 33, 1:129])

    top_list = half_closures(True)
    xsf = d["xstack_d"].ap().rearrange("c h w -> c (h w)")
    scr0 = ops.tile([1, 1], f32, tag="scr0")
    scr1 = ops.tile([1, 1], f32, tag="scr1")
    for kb in range(NCHUNK // 4):
        if kb == 1:
            # preload the sigmoid table once the first fills are queued
            nc.vector.memset(scr0[:], 0)
            nc.scalar.activation(scr1[:], scr0[:], AF.Sigmoid)
        xst = ring.tile([2 * C, 4 * CHUNK], f16, tag="xst")
        nc.sync.dma_start(xst[:], xsf[:, kb * 4 * CHUNK:(kb + 1) * 4 * CHUNK])
        rb = 16 * kb
        # conv operand filled from the bf16 copy of x straight from DRAM
        xbsrc = d["xbf_d"].ap().rearrange("c h w -> c h w")[:, rb:rb + 16, :]
        nc.scalar.dma_start(x2v[0:C, rb + 1:rb + 17, 1:129], xbsrc)
        nc.gpsimd.dma_start(x2v[C:2 * C, rb:rb + 16, 1:129], xbsrc)
        for j in range(4):
            k = 4 * kb + j
            r0 = 4 * k + 1
            psm = psM.tile([128, CHUNK], f32, tag="m")
            po = psm[0:64, :]
            nc.tensor.matmul(po, mw64[:], xst[:, j * CHUNK:(j + 1) * CHUNK],
                             start=True, stop=True, tile_position=(0, 0))
            # B-terms sit at partitions 32..49 (PSUM reads must start
            # 32-aligned); only one PSUM operand allowed per DVE op, so
            # stage them through SBUF on the Act engine.
            sb18 = stg.tile([18, CHUNK], f32, tag="sb18")
            nc.scalar.copy(sb18[:], psm[32:50, :])
            nc.vector.tensor_add(sv[:, r0:r0 + 4, 1:129],
                                 psm[0:18, :].rearrange("p (r c) -> p r c", c=W),
                                 sb18[:].rearrange("p (r c) -> p r c", c=W))
            if k == 24:
                # strip rows 0..97 final -> top-piece mask assembly can start
                maps_half(True)
            if k >= 25 and top_list:
                top_list.pop(0)()
            if k >= 2:
                conv1_chunk(k - 2)
    conv1_chunk(NCHUNK - 2)
    conv1_chunk(NCHUNK - 1)
    nc.gpsimd.dma_start(o2v[C:2 * C, 96:128, 1:129],
                        o2v[0:C, 97:129, 1:129])
    while top_list:
        top_list.pop(0)()

    # top-half gates can run during the conv1 tail / before conv2
    for s in range(3):
        gate_slab(s)

    # =====================================================================
    # Phase 2/3: bottom-half assembly overlapped with conv2 + evict
    # =====================================================================
    maps_half(False)
    bot_list = half_closures(False)

    # out pair view: partition u*64+c, free (j, w) -> c*NPOS + (2j+u)*512 + w
    outf2 = d["out_d"].ap().rearrange("c h w -> c (h w)").rearrange(
        "c (j u w) -> u c j w", u=2, w=CHUNK)

    # slab s gated before conv2 chunk (8s-5)/4 needs it; slabs >= 8 need
    # the bottom-half nm rows (ready ~chunk 8); spread 1-per-2-chunks to
    # keep the DVE gate muls from spiking
    SLAB_AT = {0: 3, 2: 4, 4: 5, 6: 6, 8: 7, 10: 8, 12: 9, 14: 10, 16: 11,
               18: 12, 20: 13, 22: 14, 24: 15}

    pB = None
    nm2b = None
    ost = None
    for k in range(NCHUNK):
        if bot_list:
            bot_list.pop(0)()
            if k < 4 and bot_list:
                bot_list.pop(0)()
        if k in SLAB_AT:
            gate_slab(SLAB_AT[k])
        par = k % 2
        if par == 0:
            # bf16 eviction sum: all-2-byte operands give the residual
            # adds the DVE 2x mode (keeps DVE under the PE budget)
            ost = stg.tile([2 * C, CHUNK], bf, tag="ost")
            ost32 = stg.tile([2 * C, CHUNK], f32, tag="ost32")
            pB = psA.tile([2 * C, CHUNK], f32, tag="cv")
            # nm2 pair broadcast: partitions 0-63 = chunk k, 64-127 = k+1
            nm2b = stg.tile([2 * C, CHUNK], bf, tag="nm2b")
            sl = nmd[0:1, k * CHUNK:(k + 1) * CHUNK]
            bsrc = bass.AP(sl.tensor, sl.offset,
                           [[CHUNK, 2], [0, C]] + list(sl.ap[1:]))
            (nc.sync, nc.scalar)[(k // 2) % 2].dma_start(nm2b[:], bsrc)
        conv_chunk(o2v, w2p, w2s, pB, par, k)
        if par == 1:
            j = k // 2
            # (conv2 + bn-bias) * nm2 in one DVE op straight from PSUM
            nc.vector.scalar_tensor_tensor(ost[:], pB[:], b2dup[:], nm2b[:],
                                           op0=ALU.add, op1=ALU.mult)
            # residual (bf16 xhi) read straight from x2's two halves:
            # lower half slot r+1 = row r (even chunk), upper slot r = row r
            # (both on DVE: gpsimd elementwise mis-addresses strided APs)
            ke = k - 1
            nc.vector.tensor_add(ost[0:C, :], ost[0:C, :],
                                 x2v[0:C, 4 * ke + 1:4 * ke + 5, 1:129])
            nc.vector.tensor_add(ost[C:2 * C, :], ost[C:2 * C, :],
                                 x2v[C:2 * C, 4 * k:4 * k + 4, 1:129])
            nc.scalar.activation(ost32[:], ost[:], AF.Relu)
            eng = (nc.sync, nc.scalar)[j % 2]
            eng.dma_start(outf2[:, :, j, :], ost32[:])

    if "dbg_o2" in d:
        nc.sync.dma_start(d["dbg_o2"].ap(), o2[:])
        nc.sync.dma_start(d["dbg_nm"].ap()[0:1, :], nmd[1:2, :])
        nc.sync.dma_start(d["dbg_nm"].ap()[1:2, :], nmd[0:1, :])
        nc.sync.dma_start(d["dbg_logit"].ap(), logit[:])

    ctx.close()


def _host_prep(inputs):
    """Fold BN, split dtypes, build packed weight blobs."""
    f32 = np.float32
    x = np.asarray(inputs["x"], f32)
    N = x.shape[0]

    def fold(w, gamma, beta, mean, var):
        scale = (np.asarray(gamma, f32)
                 / np.sqrt(np.asarray(var, f32) + f32(EPS_BN)))
        wf = np.asarray(w, f32) * scale[:, None, None, None]
        b = np.asarray(beta, f32) - np.asarray(mean, f32) * scale
        return wf, b

    w1f, b1 = fold(inputs["conv1_w"], inputs["bn1_gamma"], inputs["bn1_beta"],
                   inputs["bn1_mean"], inputs["bn1_var"])
    w2f, b2 = fold(inputs["conv2_w"], inputs["bn2_gamma"], inputs["bn2_beta"],
                   inputs["bn2_mean"], inputs["bn2_var"])

    def pack_conv(wf):
        wp = np.zeros((3, 2 * C, C), f32)
        wsg = np.zeros((3, 2 * C, C), f32)
        for kx in range(3):
            wp[kx, 0:C] = wf[:, :, 0, kx].T      # ky=0 (dy=-1), lower half
            wp[kx, C:2 * C] = wf[:, :, 1, kx].T  # ky=1 (dy=0) via shifted dup
            wsg[kx, 0:C] = wf[:, :, 2, kx].T     # ky=2 (dy=+1); upper half 0
        # [3, 2C, C] -> [2C, 3C]
        return (wp.transpose(1, 0, 2).reshape(2 * C, 3 * C),
                wsg.transpose(1, 0, 2).reshape(2 * C, 3 * C))

    w1p, w1s = pack_conv(w1f)
    w2p, w2s = pack_conv(w2f)

    mw = np.zeros((C, 18), f32)
    for t in range(9):
        ky, kx = t // 3, t % 3
        mw[:, t] = np.asarray(inputs["mask1_w"], f32)[0, :, ky, kx]
        mw[:, 9 + t] = np.asarray(inputs["mask2_w"], f32)[0, :, ky, kx]
    mwhi = mw.astype(F16).astype(f32)
    mwlo = (mw - mwhi).astype(F16).astype(f32)
    mw64 = np.zeros((2 * C, 64), f32)
    mw64[0:C, 0:18] = mwhi
    mw64[C:2 * C, 0:18] = mwlo
    mw64[0:C, 32:50] = mwlo
    mw64[C:2 * C, 32:50] = mwhi
    mw64 = mw64.astype(F16)

    syh = np.zeros((2 * C, 2 * H), f32)
    grat = np.zeros((2, H), f32)
    for mi, sig in enumerate((inputs["sigma1"], inputs["sigma2"])):
        s = f32(np.asarray(sig, f32).reshape(-1)[0])
        dd = np.arange(3, dtype=f32) - f32(1.0)
        e = np.exp(-(dd * dd) / (2 * s * s)).astype(f32)
        g1d = (e / e.sum()).astype(f32)
        g0, g1 = g1d[1], g1d[0]
        sy = np.zeros((H, H), f32)
        for h in range(H):
            for dyy in (-1, 0, 1):
                hh = h + dyy
                if 0 <= hh < H:
                    sy[hh, h] = g0 * g1d[dyy + 1]   # lhsT[h_in, h_out]
        syh[:, mi * H:(mi + 1) * H] = sy
        grat[mi, :] = g1 / g0

    wb = np.zeros((2 * C, WB_COLS), f32)
    wb[:, WB_W1P:WB_W1P + 3 * C] = w1p
    wb[:, WB_W1S:WB_W1S + 3 * C] = w1s
    wb[:, WB_W2P:WB_W2P + 3 * C] = w2p
    wb[:, WB_W2S:WB_W2S + 3 * C] = w2s

    wb[0, WB_SEL:WB_SEL + C] = 1.0
    wb[:, WB_SY:WB_SY + 2 * H] = syh
    wb = wb.astype(BF16)

    fb = np.zeros((2 * C, 8), f32)
    fb[:, 0] = np.concatenate([b1, b1])
    fb[:, 1] = np.concatenate([b2, b2])
    fb[:, 2] = np.asarray(inputs["mask1_b"], f32).reshape(-1)[0]
    fb[:, 3] = np.asarray(inputs["mask2_b"], f32).reshape(-1)[0]
    fb[0:H, 4] = grat[0]
    fb[0:H, 5] = grat[1]

    per_core = []
    for i in range(N):
        xi = x[i]
        xhi = xi.astype(F16)
        xlo = (xi - xhi.astype(f32)).astype(F16)
        xstack = np.concatenate([xhi, xlo], axis=0)
        per_core.append(dict(
            xstack=xstack,
            xbf=xi.astype(BF16),
            mw=mw64,
            wb=wb, fb=fb,
        ))
    return per_core


def kernel(**inputs):
    global _COMPILED
    if _COMPILED is None:
        _COMPILED = _build()
    nc = _COMPILED
    per_core = _host_prep(inputs)
    res = bass_utils.run_bass_kernel_spmd(
        nc, per_core, core_ids=list(range(len(per_core))))
    out = np.stack([res.results[i]["out"] for i in range(len(per_core))])
    return out.astype(np.float32)


if __name__ == "__main__":
    sys.path.insert(0, os.path.dirname(os.path.abspath(__file__)))
    import jax
    import reference
    with jax.default_device(jax.devices("cpu")[0]):
        ins = {k: np.asarray(v) for k, v in reference.setup_inputs().items()}
        exp = np.asarray(reference.reference(**ins))
    act = kernel(**ins)
    err = np.abs(act - exp)
    denom = np.abs(exp).max()
    print(f"max abs err: {err.max():.3e}  (ref scale {denom:.3f})")
    print(f"Relative error: {err.max() / denom:.3e}")


# revision 97
# speedup vs baseline: 1.0940x; 1.0369x over previous
"""Trainium2 Bass kernel for nn_BasicBlock_81166291960009.

Spatially-gated residual BasicBlock (topk_masking):
  logit_i = conv(x, mask_i_w) + mask_i_b        (64->1ch, 3x3)
  m_i = sigmoid(logit_i) * (logit_i >= 0)
  nm_i = gauss3x3(m_i, sigma_i)
  out1 = relu(bn1(conv1(x))) * nm1
  out  = relu(bn2(conv2(out1)) * nm2 + x)

Sharding: data-parallel, one sample per NeuronCore (N=8 over 8 cores),
weights replicated. No cross-core communication.

V3 layout (per core, C=64, H=W=128):
 - x streamed once: ring batches feed (a) x2 padded conv operand
   ([xhi row-shifted ; xhi], 130x130 zero border) and (b) the exact
   mask tap matmuls: ONE M=36 matmul per 512-pos chunk (lhsT packs
   [mwhi;mwlo] and [mwlo;mwhi] tap columns; a DVE fold adds the two
   halves) -> fp32 tap strip.  conv1 (6 bf16 matmuls/chunk, fp32 PSUM)
   runs in the same pass, evicted UNGATED (relu+bn-bias) to o2 bf16.
 - mask assembly: 9 paired shift-fold DMAs (mask1+mask2 taps share
   shifts), fp32 DVE tree -> logits; thresholding uses the exact fp32
   logit sign.  Gaussian: single-bf16 vertical matmul + fp32 DVE
   horizontal; nm rows stored bf16.
 - conv2 phase: o2 gated in place quarter-ahead (Pool partition
   broadcast of nm1 + Pool multiply; row-shifted dup refreshed by DMA),
   conv2 6 matmuls/chunk + K=1 selector broadcast of nm2, eviction
   bias -> *nm2 -> +residual(fp16) -> relu -> batched output DMAs.
 - SBUF arena: one fp32 [128, NPAD] tile hosts the tap strip
   (partitions 0-17), the bf16 nm rows (bitcast, partitions 0-1 after
   the strip dies), and the fp16 residual (bitcast, partitions 64-127).
 - BN folded into conv weights/bias host-side; all weights arrive in
   2 packed DMAs.
"""

import os
import sys

for _p in ("/opt/trn_rl_repo", "/root/.axon_site/_ro/trn_rl_repo"):
    if os.path.isdir(_p) and _p not in sys.path:
        sys.path.append(_p)

import numpy as np
import ml_dtypes

import concourse.bass as bass
import concourse.bacc as bacc
import concourse.tile as tile
import concourse.mybir as mybir
from concourse import bass_utils

dt = mybir.dt
AF = mybir.ActivationFunctionType
ALU = mybir.AluOpType
BF16 = ml_dtypes.bfloat16
F16 = np.float16

C = 64            # channels
H = W = 128       # spatial
P = 130           # padded pitch
NPOS = H * W      # 16384
NPAD = P * P      # 16900
CHUNK = 512       # positions per PSUM bank (4 image rows)
NCHUNK = NPOS // CHUNK   # 32
EPS_BN = 1e-5
N_CORES = 8

# wb blob column offsets (bf16)
WB_W1P, WB_W1S, WB_W2P, WB_W2S = 0, 192, 384, 576
WB_MW, WB_SEL, WB_SY = 768, 832, 896
WB_COLS = 1152

_COMPILED = None


def _bf16_split(a):
    hi = a.astype(BF16)
    lo = (a.astype(np.float32) - hi.astype(np.float32)).astype(BF16)
    return hi, lo


def _build():
    """Build + compile the single-core Bass module (shapes fixed)."""
    nc = bacc.Bacc("TRN2", target_bir_lowering=False, debug=False,
                   num_devices=N_CORES)

    f32, bf = dt.float32, dt.bfloat16

    xstack_d = nc.dram_tensor("xstack", [2 * C, H, W], dt.float16,
                              kind="ExternalInput")
    xbf_d = nc.dram_tensor("xbf", [C, H, W], bf, kind="ExternalInput")
    mw_d = nc.dram_tensor("mw", [2 * C, 64], dt.float16,
                          kind="ExternalInput")
    wb_d = nc.dram_tensor("wb", [2 * C, WB_COLS], bf, kind="ExternalInput")
    fb_d = nc.dram_tensor("fb", [2 * C, 8], f32, kind="ExternalInput")
    out_d = nc.dram_tensor("out", [C, H, W], f32, kind="ExternalOutput")
    # nm rows staged in DRAM: broadcasts are then plain DMAs with
    # stride-0 DRAM sources (partition_broadcast races in-flight writes)
    nmd_d = nc.dram_tensor("nmd", [2, NPOS], bf, kind="Internal")
    d = dict(xstack_d=xstack_d, xbf_d=xbf_d, mw_d=mw_d, wb_d=wb_d,
             fb_d=fb_d, out_d=out_d, nmd_d=nmd_d)
    if os.environ.get("K_DEBUG") == "1":
        d["dbg_o2"] = nc.dram_tensor("dbg_o2", [2 * C, NPAD], bf,
                                     kind="ExternalOutput")
        d["dbg_nm"] = nc.dram_tensor("dbg_nm", [2, NPOS], bf,
                                     kind="ExternalOutput")
        d["dbg_logit"] = nc.dram_tensor("dbg_logit", [H, 2 * H], f32,
                                        kind="ExternalOutput")

    with tile.TileContext(nc) as tc:
        _emit(nc, tc, d)
    nc.compile()
    return nc


def _emit(nc, tc, d):
    f32, bf, f16 = dt.float32, dt.bfloat16, dt.float16
    from contextlib import ExitStack
    ctx = ExitStack()

    big = ctx.enter_context(tc.tile_pool(name="big", bufs=1))
    wts = ctx.enter_context(tc.tile_pool(name="wts", bufs=1))
    ring = ctx.enter_context(tc.tile_pool(name="ring", bufs=2))
    stg = ctx.enter_context(tc.tile_pool(name="stg", bufs=2))
    ops = ctx.enter_context(tc.tile_pool(name="ops", bufs=1))
    psA = ctx.enter_context(tc.tile_pool(name="psA", bufs=4, space="PSUM"))
    psM = ctx.enter_context(tc.tile_pool(name="psM", bufs=2, space="PSUM"))

    # ---- persistent tiles ----
    x2 = big.tile([2 * C, NPAD], bf, tag="x2")        # padded x + shifted dup
    o2 = big.tile([2 * C, NPAD], bf, tag="o2")        # padded out1 + dup
    arena = big.tile([2 * C, NPAD], f32, tag="arena")
    maps = big.tile([H, 9 * 2 * H], f32, tag="maps")  # [128h, 9tap, 2mask, 128w]

    strip = arena[0:18, :]
    nmd = d["nmd_d"].ap()   # row 0 = nm2, row 1 = nm1 (DRAM)

    wb = wts.tile([2 * C, WB_COLS], bf, tag="wb")
    mw64t = wts.tile([2 * C, 64], f16, tag="mw64t")
    nc.sync.dma_start(mw64t[:], d["mw_d"].ap())
    fb = wts.tile([2 * C, 8], f32, tag="fb")

    w1p = wb[:, WB_W1P:WB_W1P + 3 * C]
    w1s = wb[:, WB_W1S:WB_W1S + 3 * C]
    w2p = wb[:, WB_W2P:WB_W2P + 3 * C]
    w2s = wb[:, WB_W2S:WB_W2S + 3 * C]
    mw64 = mw64t[:, :]
    sel1 = wb[0:1, WB_SEL:WB_SEL + C]
    syh = wb[:, WB_SY:WB_SY + 2 * H]
    b1dup = fb[:, 0:1]
    b2dup = fb[:, 1:2]
    mb = fb[:, 2:4]
    grat = fb[:, 4:6]

    nc.scalar.dma_start(wb[:], d["wb_d"].ap())
    nc.scalar.dma_start(fb[:], d["fb_d"].ap())

    x2v = x2.rearrange("p (r c) -> p r c", c=P)
    o2v = o2.rearrange("p (r c) -> p r c", c=P)
    sv = strip.rearrange("p (r c) -> p r c", c=P)
    mapsv = maps.rearrange("p (t m c) -> p t m c", t=9, m=2)

    # ---- pad memsets (zero borders) ----
    for tv in (x2v, o2v):
        nc.vector.memset(tv[:, 0, :], 0)
        nc.vector.memset(tv[:, P - 1, :], 0)
        nc.vector.memset(tv[:, 1:P - 1, 0:1], 0)
        nc.vector.memset(tv[:, 1:P - 1, P - 1:P], 0)
        nc.vector.memset(tv[C:2 * C, P - 2, :], 0)
    nc.vector.memset(sv[:, 0, :], 0)
    nc.vector.memset(sv[:, P - 1, :], 0)
    nc.vector.memset(sv[:, 1:P - 1, 0:1], 0)
    nc.vector.memset(sv[:, 1:P - 1, P - 1:P], 0)

    # residual: loaded in 4 pieces mid-conv1 (keeps DMA engines free early)

    # ---- mask-assembly tiles; each is written in two row-halves ----
    # (the tap tree accumulates in place inside `maps`)
    logit = ops.tile([H, 2 * H], f32, tag="logit")
    logitv = logit.rearrange("p (m c) -> p m c", m=2)
    pp = ops.tile([H, 2 * H], f32, tag="pp")
    mbf = ops.tile([H, 2 * H], bf, tag="mbf")
    mbfv = mbf.rearrange("p (m c) -> p m c", m=2)
    nmv = ops.tile([H, 2 * P], f32, tag="nmv")
    nmvv = nmv.rearrange("p (m c) -> p m c", m=2)
    t2 = ops.tile([H, 2 * H], f32, tag="t2")
    t2v = t2.rearrange("p (m c) -> p m c", m=2)
    nmf = ops.tile([H, 2 * H], bf, tag="nmf")
    nmfv = nmf.rearrange("p (m c) -> p m c", m=2)
    # the K=128 vertical-gauss of each half multiplies the other half's
    # (possibly unwritten) mbf rows by zero sy-weights; keep them finite
    nc.vector.memset(mbf[:], 0)
    nc.vector.memset(nmvv[:, :, 0:1], 0)
    nc.vector.memset(nmvv[:, :, P - 1:P], 0)

    engs = (nc.sync, nc.scalar, nc.gpsimd)

    def maps_half(top):
        # 3-way queue split: the Pool queue is otherwise idle here (the nm
        # broadcasts are HWDGE DMAs now), so gpsimd SWDGE absorbs a third
        # of the setup cost that used to serialize on HWDGE
        r0, r1 = (0, 96) if top else (96, 128)
        for t in range(18):
            t9, mi = t % 9, t // 9
            dy, dx = t9 // 3 - 1, t9 % 3 - 1
            src = sv[9 * mi + t9:9 * mi + t9 + 1,
                     1 + dy + r0:1 + dy + r1, 1 + dx:129 + dx]
            engs[t % 3].dma_start(mapsv[r0:r1, t9, mi, :], src)

    def half_closures(top):
        # logit/m rows split at 96 (SBUF partition bases must be 32-aligned);
        # gauss/nm output rows split at 64 (each K=128 gauss reads the m rows
        # it needs from either piece)
        r0, r1 = (0, 96) if top else (96, 128)
        g0, g1 = (0, C) if top else (C, 2 * C)
        box = {}

        def op_u1():
            nc.vector.tensor_add(maps[r0:r1, 0:4 * 2 * H],
                                 maps[r0:r1, 0:4 * 2 * H],
                                 maps[r0:r1, 4 * 2 * H:8 * 2 * H])

        def op_u2():
            nc.vector.tensor_add(maps[r0:r1, 0:2 * 2 * H],
                                 maps[r0:r1, 0:2 * 2 * H],
                                 maps[r0:r1, 2 * 2 * H:4 * 2 * H])

        def op_u3():
            nc.vector.tensor_add(maps[r0:r1, 0:2 * H],
                                 maps[r0:r1, 0:2 * H],
                                 maps[r0:r1, 2 * H:4 * H])

        def op_logit():
            for mi in range(2):
                nc.vector.scalar_tensor_tensor(
                    logitv[r0:r1, mi, :], mapsv[r0:r1, 0, mi, :],
                    mb[r0:r1, mi:mi + 1], mapsv[r0:r1, 8, mi, :],
                    op0=ALU.add, op1=ALU.add)

        def op_sig():
            nc.scalar.activation(pp[r0:r1, :], logit[r0:r1, :], AF.Sigmoid)

        def op_m():
            # hard mask in place over the sigmoid values
            nc.vector.scalar_tensor_tensor(
                pp[r0:r1, :], logit[r0:r1, :], 0.0, pp[r0:r1, :],
                op0=ALU.is_ge, op1=ALU.mult)

        def op_mbf():
            nc.vector.tensor_copy(mbf[r0:r1, :], pp[r0:r1, :])

        def op_gauss():
            png = psM.tile([128, CHUNK], f32, tag="b")
            pngv = png[:, 0:2 * H].rearrange("p (m c) -> p m c", m=2)
            box["pngv"] = pngv
            for mi in range(2):
                nc.tensor.matmul(pngv[g0:g1, mi, :],
                                 syh[:, mi * H + g0:mi * H + g1],
                                 mbfv[:, mi, :], start=True, stop=True)

        def op_nmvc():
            nc.scalar.copy(nmvv[g0:g1, :, 1:129], box["pngv"][g0:g1, :, :])

        def op_t2():
            nc.vector.tensor_add(t2v[g0:g1, :, :], nmvv[g0:g1, :, 0:128],
                                 nmvv[g0:g1, :, 2:130])

        def op_nm():
            for mi in range(2):
                nc.vector.scalar_tensor_tensor(
                    nmfv[g0:g1, mi, :], t2v[g0:g1, mi, :],
                    grat[g0:g1, mi:mi + 1], nmvv[g0:g1, mi, 1:129],
                    op0=ALU.mult, op1=ALU.add)

        def op_rows():
            nv = nmd.rearrange("p (h w) -> p h w", w=W)
            nc.sync.dma_start(nv[0:1, g0:g1, :], nmfv[g0:g1, 1, :])
            nc.scalar.dma_start(nv[1:2, g0:g1, :], nmfv[g0:g1, 0, :])

        return [op_u1, op_u2, op_u3, op_logit, op_sig, op_m, op_mbf,
                op_gauss, op_nmvc, op_t2, op_nm, op_rows]

    def gate_slab(s):
        # gates chunks 2s, 2s+1 (rows 8s..8s+7); nm1 broadcast to all 128
        # partitions by a stride-0-source DMA from DRAM
        nm1b = stg.tile([2 * C, 2 * CHUNK], bf, tag="nm1b")
        sl = nmd[1:2, s * 2 * CHUNK:(s + 1) * 2 * CHUNK]
        bsrc = bass.AP(sl.tensor, sl.offset, [[0, 2 * C]] + list(sl.ap[1:]))
        (nc.sync, nc.scalar)[s % 2].dma_start(nm1b[:], bsrc)
        nmv8lo = nm1b[0:C, :].rearrange("p (r c) -> p r c", c=W)
        nmv8up = nm1b[C:2 * C, :].rearrange("p (r c) -> p r c", c=W)
        r0 = 8 * s
        lo = o2v[0:C, r0 + 1:r0 + 9, 1:129]
        nc.vector.tensor_mul(lo, lo, nmv8lo)
        up = o2v[C:2 * C, r0:r0 + 8, 1:129]
        nc.vector.tensor_mul(up, up, nmv8up)

    # =====================================================================
    # conv helper (baseline-proven indexing)
    # =====================================================================
    def conv_chunk(src2v, wp, ws, ps, par, k):
        tp = (0, 0) if par == 0 else (0, 64)
        po = ps[0:C, :] if par == 0 else ps[C:2 * C, :]
        r0 = 4 * k + 1
        for kx in range(3):
            dx = kx - 1
            rhs = src2v[:, r0 - 1:r0 + 3, 1 + dx:129 + dx]
            nc.tensor.matmul(po, wp[:, kx * C:(kx + 1) * C], rhs,
                             start=(kx == 0), stop=False, tile_position=tp)
            rhs1 = src2v[:, r0 + 1:r0 + 5, 1 + dx:129 + dx]
            nc.tensor.matmul(po, ws[:, kx * C:(kx + 1) * C], rhs1,
                             start=False, stop=(kx == 2), tile_position=tp)

    # =====================================================================
    # Phase 1: stream x; mask taps (1 matmul/chunk) + conv1 (ungated)
    # =====================================================================
    pA = None

    def conv1_chunk(k):
        nonlocal pA
        par = k % 2
        if par == 0:
            pA = psA.tile([2 * C, CHUNK], f32, tag="cv")
        conv_chunk(x2v, w1p, w1s, pA, par, k)
        if par == 1:
            for pr in range(2):
                kk = k - 1 + pr
                rr = 4 * kk + 1
                h0, h1 = (0, C) if pr == 0 else (C, 2 * C)
                dst = o2v[0:C, rr:rr + 4, 1:129]
                nc.scalar.activation(dst, pA[h0:h1, :], AF.Relu,
                                     bias=b1dup[h0:h1, :])
        # refresh the row-shifted dup (UNGATED; gating hits both halves
        # later) as soon as a 32-row quarter is fully evicted
        if k >= 10 and (k - 10) % 8 == 0:
            q = (k - 10) // 8
            nc.gpsimd.dma_start(o2v[C:2 * C, 32 * q:32 * q + 32, 1:129],
                                o2v[0:C, 32 * q + 1:32 * q + 33, 1:129])

    top_list = half_closures(True)
    xsf = d["xstack_d"].ap().rearrange("c h w -> c (h w)")
    scr0 = ops.tile([1, 1], f32, tag="scr0")
    scr1 = ops.tile([1, 1], f32, tag="scr1")
    for kb in range(NCHUNK // 4):
        if kb == 1:
            # preload the sigmoid table once the first fills are queued
            nc.vector.memset(scr0[:], 0)
            nc.scalar.activation(scr1[:], scr0[:], AF.Sigmoid)
        xst = ring.tile([2 * C, 4 * CHUNK], f16, tag="xst")
        nc.sync.dma_start(xst[:], xsf[:, kb * 4 * CHUNK:(kb + 1) * 4 * CHUNK])
        rb = 16 * kb
        # conv operand filled from the bf16 copy of x straight from DRAM
        xbsrc = d["xbf_d"].ap().rearrange("c h w -> c h w")[:, rb:rb + 16, :]
        nc.scalar.dma_start(x2v[0:C, rb + 1:rb + 17, 1:129], xbsrc)
        nc.gpsimd.dma_start(x2v[C:2 * C, rb:rb + 16, 1:129], xbsrc)
        for j in range(4):
            k = 4 * kb + j
            r0 = 4 * k + 1
            psm = psM.tile([128, CHUNK], f32, tag="m")
            po = psm[0:64, :]
            nc.tensor.matmul(po, mw64[:], xst[:, j * CHUNK:(j + 1) * CHUNK],
                             start=True, stop=True, tile_position=(0, 0))
            # B-terms sit at partitions 32..49 (PSUM reads must start
            # 32-aligned); only one PSUM operand allowed per DVE op, so
            # stage them through SBUF on the Act engine.
            sb18 = stg.tile([18, CHUNK], f32, tag="sb18")
            nc.scalar.copy(sb18[:], psm[32:50, :])
            nc.vector.tensor_add(sv[:, r0:r0 + 4, 1:129],
                                 psm[0:18, :].rearrange("p (r c) -> p r c", c=W),
                                 sb18[:].rearrange("p (r c) -> p r c", c=W))
            if k == 24:
                # strip rows 0..97 final -> top-piece mask assembly can start
                maps_half(True)
            if k >= 25 and top_list:
                top_list.pop(0)()
            if k >= 2:
                conv1_chunk(k - 2)
    conv1_chunk(NCHUNK - 2)
    conv1_chunk(NCHUNK - 1)
    nc.gpsimd.dma_start(o2v[C:2 * C, 96:128, 1:129],
                        o2v[0:C, 97:129, 1:129])
    while top_list:
        top_list.pop(0)()

    # top-half gates can run during the conv1 tail / before conv2
    for s in range(3):
        gate_slab(s)

    # =====================================================================
    # Phase 2/3: bottom-half assembly overlapped with conv2 + evict
    # =====================================================================
    maps_half(False)
    bot_list = half_closures(False)

    # out pair view: partition u*64+c, free (j, w) -> c*NPOS + (2j+u)*512 + w
    outf2 = d["out_d"].ap().rearrange("c h w -> c (h w)").rearrange(
        "c (j u w) -> u c j w", u=2, w=CHUNK)

    # slab s gated before conv2 chunk (8s-5)/4 needs it; slabs >= 8 need
    # the bottom-half nm rows (ready ~chunk 8); spread 1-per-2-chunks to
    # keep the DVE gate muls from spiking
    SLAB_AT = {0: 3, 2: 4, 4: 5, 6: 6, 8: 7, 10: 8, 12: 9, 14: 10, 16: 11,
               18: 12, 20: 13, 22: 14, 24: 15}

    pB = None
    nm2b = None
    ost = None
    for k in range(NCHUNK):
        if bot_list:
            bot_list.pop(0)()
            if k < 4 and bot_list:
                bot_list.pop(0)()
        if k in SLAB_AT:
            gate_slab(SLAB_AT[k])
        par = k % 2
        if par == 0:
            # bf16 eviction sum: all-2-byte operands give the residual
            # adds the DVE 2x mode (keeps DVE under the PE budget)
            ost = stg.tile([2 * C, CHUNK], bf, tag="ost")
            ost32 = stg.tile([2 * C, CHUNK], f32, tag="ost32")
            pB = psA.tile([2 * C, CHUNK], f32, tag="cv")
            # nm2 pair broadcast: partitions 0-63 = chunk k, 64-127 = k+1
            nm2b = stg.tile([2 * C, CHUNK], bf, tag="nm2b")
            sl = nmd[0:1, k * CHUNK:(k + 1) * CHUNK]
            bsrc = bass.AP(sl.tensor, sl.offset,
                           [[CHUNK, 2], [0, C]] + list(sl.ap[1:]))
            (nc.sync, nc.scalar)[(k // 2) % 2].dma_start(nm2b[:], bsrc)
        conv_chunk(o2v, w2p, w2s, pB, par, k)
        if par == 1:
            j = k // 2
            # (conv2 + bn-bias) * nm2 in one DVE op straight from PSUM
            nc.vector.scalar_tensor_tensor(ost[:], pB[:], b2dup[:], nm2b[:],
                                           op0=ALU.add, op1=ALU.mult)
            # residual (bf16 xhi) read straight from x2's two halves:
            # lower half slot r+1 = row r (even chunk), upper slot r = row r
            # (both on DVE: gpsimd elementwise mis-addresses strided APs)
            ke = k - 1
            nc.vector.tensor_add(ost[0:C, :], ost[0:C, :],
                                 x2v[0:C, 4 * ke + 1:4 * ke + 5, 1:129])
            nc.vector.tensor_add(ost[C:2 * C, :], ost[C:2 * C, :],
                                 x2v[C:2 * C, 4 * k:4 * k + 4, 1:129])
            nc.scalar.activation(ost32[:], ost[:], AF.Relu)
            eng = (nc.sync, nc.scalar)[j % 2]
            eng.dma_start(outf2[:, :, j, :], ost32[:])

    if "dbg_o2" in d:
        nc.sync.dma_start(d["dbg_o2"].ap(), o2[:])
        nc.sync.dma_start(d["dbg_nm"].ap()[0:1, :], nmd[1:2, :])
        nc.sync.dma_start(d["dbg_nm"].ap()[1:2, :], nmd[0:1, :])
        nc.sync.dma_start(d["dbg_logit"].ap(), logit[:])

    ctx.close()


def _host_prep(inputs):
    """Fold BN, split dtypes, build packed weight blobs."""
    f32 = np.float32
    x = np.asarray(inputs["x"], f32)
    N = x.shape[0]

    def fold(w, gamma, beta, mean, var):
        scale = (np.asarray(gamma, f32)
                 / np.sqrt(np.asarray(var, f32) + f32(EPS_BN)))
        wf = np.asarray(w, f32) * scale[:, None, None, None]
        b = np.asarray(beta, f32) - np.asarray(mean, f32) * scale
        return wf, b

    w1f, b1 = fold(inputs["conv1_w"], inputs["bn1_gamma"], inputs["bn1_beta"],
                   inputs["bn1_mean"], inputs["bn1_var"])
    w2f, b2 = fold(inputs["conv2_w"], inputs["bn2_gamma"], inputs["bn2_beta"],
                   inputs["bn2_mean"], inputs["bn2_var"])

    def pack_conv(wf):
        wp = np.zeros((3, 2 * C, C), f32)
        wsg = np.zeros((3, 2 * C, C), f32)
        for kx in range(3):
            wp[kx, 0:C] = wf[:, :, 0, kx].T      # ky=0 (dy=-1), lower half
            wp[kx, C:2 * C] = wf[:, :, 1, kx].T  # ky=1 (dy=0) via shifted dup
            wsg[kx, 0:C] = wf[:, :, 2, kx].T     # ky=2 (dy=+1); upper half 0
        # [3, 2C, C] -> [2C, 3C]
        return (wp.transpose(1, 0, 2).reshape(2 * C, 3 * C),
                wsg.transpose(1, 0, 2).reshape(2 * C, 3 * C))

    w1p, w1s = pack_conv(w1f)
    w2p, w2s = pack_conv(w2f)

    mw = np.zeros((C, 18), f32)
    for t in range(9):
        ky, kx = t // 3, t % 3
        mw[:, t] = np.asarray(inputs["mask1_w"], f32)[0, :, ky, kx]
        mw[:, 9 + t] = np.asarray(inputs["mask2_w"], f32)[0, :, ky, kx]
    mwhi = mw.astype(F16).astype(f32)
    mwlo = (mw - mwhi).astype(F16).astype(f32)
    mw64 = np.zeros((2 * C, 64), f32)
    mw64[0:C, 0:18] = mwhi
    mw64[C:2 * C, 0:18] = mwlo
    mw64[0:C, 32:50] = mwlo
    mw64[C:2 * C, 32:50] = mwhi
    mw64 = mw64.astype(F16)

    syh = np.zeros((2 * C, 2 * H), f32)
    grat = np.zeros((2, H), f32)
    for mi, sig in enumerate((inputs["sigma1"], inputs["sigma2"])):
        s = f32(np.asarray(sig, f32).reshape(-1)[0])
        dd = np.arange(3, dtype=f32) - f32(1.0)
        e = np.exp(-(dd * dd) / (2 * s * s)).astype(f32)
        g1d = (e / e.sum()).astype(f32)
        g0, g1 = g1d[1], g1d[0]
        sy = np.zeros((H, H), f32)
        for h in range(H):
            for dyy in (-1, 0, 1):
                hh = h + dyy
                if 0 <= hh < H:
                    sy[hh, h] = g0 * g1d[dyy + 1]   # lhsT[h_in, h_out]
        syh[:, mi * H:(mi + 1) * H] = sy
        grat[mi, :] = g1 / g0

    wb = np.zeros((2 * C, WB_COLS), f32)
    wb[:, WB_W1P:WB_W1P + 3 * C] = w1p
    wb[:, WB_W1S:WB_W1S + 3 * C] = w1s
    wb[:, WB_W2P:WB_W2P + 3 * C] = w2p
    wb[:, WB_W2S:WB_W2S + 3 * C] = w2s

    wb[0, WB_SEL:WB_SEL + C] = 1.0
    wb[:, WB_SY:WB_SY + 2 * H] = syh
    wb = wb.astype(BF16)

    fb = np.zeros((2 * C, 8), f32)
    fb[:, 0] = np.concatenate([b1, b1])
    fb[:, 1] = np.concatenate([b2, b2])
    fb[:, 2] = np.asarray(inputs["mask1_b"], f32).reshape(-1)[0]
    fb[:, 3] = np.asarray(inputs["mask2_b"], f32).reshape(-1)[0]
    fb[0:H, 4] = grat[0]
    fb[0:H, 5] = grat[1]

    per_core = []
    for i in range(N):
        xi = x[i]
        xhi = xi.astype(F16)
        xlo = (xi - xhi.astype(f32)).astype(F16)
        xstack = np.concatenate([xhi, xlo], axis=0)
        per_core.append(dict(
            xstack=xstack,
            xbf=xi.astype(BF16),
            mw=mw64,
            wb=wb, fb=fb,
        ))
    return per_core


def kernel(**inputs):
    global _COMPILED
    if _COMPILED is None:
        _COMPILED = _build()
    nc = _COMPILED
    per_core = _host_prep(inputs)
    res = bass_utils.run_bass_kernel_spmd(
        nc, per_core, core_ids=list(range(len(per_core))))
    out = np.stack([res.results[i]["out"] for i in range(len(per_core))])
    return out.astype(np.float32)


if __name__ == "__main__":
    sys.path.insert(0, os.path.dirname(os.path.abspath(__file__)))
    import jax
    import reference
    with jax.default_device(jax.devices("cpu")[0]):
        ins = {k: np.asarray(v) for k, v in reference.setup_inputs().items()}
        exp = np.asarray(reference.reference(**ins))
    act = kernel(**ins)
    err = np.abs(act - exp)
    denom = np.abs(exp).max()
    print(f"max abs err: {err.max():.3e}  (ref scale {denom:.3f})")
    print(f"Relative error: {err.max() / denom:.3e}")
